# revision 7
# baseline (speedup 1.0000x reference)
"""Contrastive loss (ConStealLoss) kernel for 8 Trainium2 NeuronCores.

Computes, for x, y in R^[B=8192, D=128], T=0.07:
    sim1 = exp(x @ y.T / T); sim2 = exp(x @ x.T / T)
    l_neg_1 = rowsum(sim1) - diag(sim1); l_neg_2 = rowsum(sim2) - diag(sim2)
    l_pos = exp(rowsum(x*y) / T)
    loss = mean(-log(l_pos / (l_neg_1 + l_neg_2)))

Sharding: data-parallel over rows of x (1024 rows/core); y and x are
replicated as bf16 [D, B] "key" panels. Each core produces the partial
sum of log(l_pos/(l1+l2)) over its rows; the host combines.
"""

import numpy as np
import ml_dtypes

B = 8192
D = 128
TEMP = 0.07
NCORES = 8
R = B // NCORES            # 1024 rows per core
MT = R // 128              # 8 row-tiles of 128 per core
CHUNK = 2048               # keys per ScalarE exp instruction (4 PSUM banks)
NG = B // CHUNK            # 4 chunk groups over the key axis
NB = CHUNK // 512          # 4 matmuls (N=512) per chunk

_CACHE = {}


def _build_nc():
    import concourse.mybir as mybir
    import concourse.tile as tile
    from concourse import bacc

    f32 = mybir.dt.float32
    bf16 = mybir.dt.bfloat16
    AF = mybir.ActivationFunctionType
    AX = mybir.AxisListType
    ALU = mybir.AluOpType

    nc = bacc.Bacc("TRN2", debug=False)

    xT_d = nc.declare_dram_parameter("xT", [D, B], bf16, isOutput=False)
    yT_d = nc.declare_dram_parameter("yT", [D, B], bf16, isOutput=False)
    xcT_d = nc.declare_dram_parameter("xcT", [D, R], bf16, isOutput=False)
    xn_d = nc.declare_dram_parameter("xn", [R, D], f32, isOutput=False)
    yn_d = nc.declare_dram_parameter("yn", [R, D], f32, isOutput=False)
    out_d = nc.declare_dram_parameter("out", [128, 1], f32, isOutput=True)

    with tile.TileContext(nc) as tc:
        with (
            tc.tile_pool(name="const", bufs=1) as cpool,
            tc.tile_pool(name="scratch", bufs=3) as spool,
            tc.tile_pool(name="psum", bufs=2, space="PSUM") as ppool,
        ):
            xcT = cpool.tile([D, R], bf16)
            yT = cpool.tile([D, B], bf16)
            xT = cpool.tile([D, B], bf16)
            nc.sync.dma_start(xcT[:], xcT_d.ap())
            nc.sync.dma_start(yT[:], yT_d.ap())
            nc.sync.dma_start(xT[:], xT_d.ap())
            xns, yns = [], []
            for m in range(MT):
                xnm = cpool.tile([128, D], f32, name=f"xn_{m}", tag=f"xn_{m}")
                ynm = cpool.tile([128, D], f32, name=f"yn_{m}", tag=f"yn_{m}")
                nc.sync.dma_start(xnm[:], xn_d.ap()[m * 128:(m + 1) * 128, :])
                nc.sync.dma_start(ynm[:], yn_d.ap()[m * 128:(m + 1) * 128, :])
                xns.append(xnm)
                yns.append(ynm)

            racc = cpool.tile([128, MT * 2 * NG], f32)
            d1 = cpool.tile([128, MT], f32)
            d2 = cpool.tile([128, MT], f32)

            # Per-row dots: d1[r] = <x_r, y_r>, d2[r] = |x_r|^2 (f32, natural layout)
            for m in range(MT):
                prod = spool.tile([128, D], f32, tag="prod")
                nc.vector.tensor_mul(prod[:], xns[m][:], yns[m][:])
                nc.vector.reduce_sum(d1[:, m:m + 1], prod[:], axis=AX.X)
                prod2 = spool.tile([128, D], f32, tag="prod")
                nc.vector.tensor_mul(prod2[:], xns[m][:], xns[m][:])
                nc.vector.reduce_sum(d2[:, m:m + 1], prod2[:], axis=AX.X)

            # Main: PE matmul -> PSUM [128, 2048]; ScalarE exp with row-sum accumulate
            for m in range(MT):
                lhsT = xcT[:, m * 128:(m + 1) * 128]
                for s, keys in enumerate((yT, xT)):
                    for g in range(NG):
                        pt = ppool.tile([128, CHUNK], f32, tag="pt")
                        for b in range(NB):
                            k0 = g * CHUNK + b * 512
                            nc.tensor.matmul(
                                pt[:, b * 512:(b + 1) * 512],
                                lhsT,
                                keys[:, k0:k0 + 512],
                                start=True, stop=True,
                            )
                        es = spool.tile([128, CHUNK], bf16, tag="es")
                        col = (m * 2 + s) * NG + g
                        nc.scalar.activation(
                            es[:], pt[:], AF.Exp, scale=1.0 / TEMP,
                            accum_out=racc[:, col:col + 1],
                        )

            # Row sums per (m, sim)
            rs1 = cpool.tile([128, MT], f32)
            rs2 = cpool.tile([128, MT], f32)
            for m in range(MT):
                c0 = (m * 2) * NG
                nc.vector.reduce_sum(rs1[:, m:m + 1], racc[:, c0:c0 + NG], axis=AX.X)
                nc.vector.reduce_sum(rs2[:, m:m + 1], racc[:, c0 + NG:c0 + 2 * NG], axis=AX.X)

            # Epilogue: l1 = rs1 - exp(d1/T); l2 = rs2 - exp(d2/T);
            # lg = log(exp(d1/T) / (l1 + l2)); out = rowwise sum of lg
            diag1 = cpool.tile([128, MT], f32)
            diag2 = cpool.tile([128, MT], f32)
            nc.scalar.activation(diag1[:], d1[:], AF.Exp, scale=1.0 / TEMP)
            nc.scalar.activation(diag2[:], d2[:], AF.Exp, scale=1.0 / TEMP)
            l1 = cpool.tile([128, MT], f32)
            l2 = cpool.tile([128, MT], f32)
            nc.vector.tensor_sub(l1[:], rs1[:], diag1[:])
            nc.vector.tensor_sub(l2[:], rs2[:], diag2[:])
            ssum = cpool.tile([128, MT], f32)
            nc.vector.tensor_add(ssum[:], l1[:], l2[:])
            rec = cpool.tile([128, MT], f32)
            nc.vector.reciprocal(rec[:], ssum[:])
            ratio = cpool.tile([128, MT], f32)
            nc.vector.tensor_mul(ratio[:], diag1[:], rec[:])
            lg = cpool.tile([128, MT], f32)
            nc.scalar.activation(lg[:], ratio[:], AF.Ln)
            rowpart = cpool.tile([128, 1], f32)
            nc.vector.reduce_sum(rowpart[:], lg[:], axis=AX.X)
            nc.sync.dma_start(out_d.ap(), rowpart[:])

    nc.compile()
    return nc


def _prep_inputs(x: np.ndarray, y: np.ndarray):
    x = np.ascontiguousarray(x, dtype=np.float32)
    y = np.ascontiguousarray(y, dtype=np.float32)
    xT = np.ascontiguousarray(x.T).astype(ml_dtypes.bfloat16)
    yT = np.ascontiguousarray(y.T).astype(ml_dtypes.bfloat16)
    in_maps = []
    for c in range(NCORES):
        sl = slice(c * R, (c + 1) * R)
        in_maps.append({
            "xT": xT,
            "yT": yT,
            "xcT": np.ascontiguousarray(xT[:, sl]),
            "xn": np.ascontiguousarray(x[sl]),
            "yn": np.ascontiguousarray(y[sl]),
        })
    return in_maps


def _get_runner():
    """Build the Bacc program once and wrap it in a cached sharded jit."""
    if "runner" in _CACHE:
        return _CACHE["runner"]
    import jax
    import jax.numpy as jnp
    from jax.sharding import Mesh, PartitionSpec
    from jax.experimental.shard_map import shard_map
    import concourse.mybir as mybir
    from concourse import bass2jax

    nc = _build_nc()
    bass2jax.install_neuronx_cc_hook()

    in_names = []
    out_names = []
    out_avals = []
    partition_name = (
        nc.partition_id_tensor.name if nc.partition_id_tensor else None
    )
    for alloc in nc.m.functions[0].allocations:
        if not isinstance(alloc, mybir.MemoryLocationSet):
            continue
        name = alloc.memorylocations[0].name
        if alloc.kind == "ExternalInput":
            if name != partition_name:
                in_names.append(name)
        elif alloc.kind == "ExternalOutput":
            out_names.append(name)
            out_avals.append(
                jax.core.ShapedArray(
                    tuple(alloc.tensor_shape), mybir.dt.np(alloc.dtype)
                )
            )
    n_params = len(in_names)
    n_outs = len(out_names)
    all_names = list(in_names) + list(out_names)
    if partition_name is not None:
        all_names.append(partition_name)

    def _body(*args):
        operands = list(args)
        if partition_name is not None:
            operands.append(bass2jax.partition_id_tensor())
        outs = bass2jax._bass_exec_p.bind(
            *operands,
            out_avals=tuple(out_avals),
            in_names=tuple(all_names),
            out_names=tuple(out_names),
            lowering_input_output_aliases=(),
            sim_require_finite=False,
            sim_require_nnan=False,
            nc=nc,
        )
        return tuple(outs)

    devices = jax.devices()[:NCORES]
    mesh = Mesh(np.asarray(devices), ("core",))
    in_specs = (PartitionSpec("core"),) * (n_params + n_outs)
    out_specs = (PartitionSpec("core"),) * n_outs
    donate = tuple(range(n_params, n_params + n_outs))
    jitted = jax.jit(
        shard_map(_body, mesh=mesh, in_specs=in_specs, out_specs=out_specs,
                  check_rep=False),
        donate_argnums=donate,
        keep_unused=True,
    )
    zero_shapes = [
        (tuple(av.shape), av.dtype) for av in out_avals
    ]
    runner = {
        "jitted": jitted,
        "in_names": in_names,
        "out_names": out_names,
        "zero_shapes": zero_shapes,
        "nc": nc,
    }
    _CACHE["runner"] = runner
    return runner


def _execute(in_maps):
    r = _get_runner()
    concat_in = [
        np.concatenate([np.asarray(in_maps[c][name]) for c in range(NCORES)], axis=0)
        for name in r["in_names"]
    ]
    zeros = [np.zeros((NCORES * s[0],) + tuple(s[1:]), dt)
             for s, dt in r["zero_shapes"]]
    outs = r["jitted"](*concat_in, *zeros)
    results = []
    for c in range(NCORES):
        m = {}
        for i, name in enumerate(r["out_names"]):
            arr = np.asarray(outs[i])
            per = arr.shape[0] // NCORES
            m[name] = arr[c * per:(c + 1) * per]
        results.append(m)
    return results


def _run(x: np.ndarray, y: np.ndarray, trace: bool = False):
    in_maps = _prep_inputs(x, y)
    results = _execute(in_maps)
    total = np.float32(0.0)
    for c in range(NCORES):
        total = total + results[c]["out"].astype(np.float32).sum()
    loss = np.float32(-(total / np.float32(B)))
    return loss, results


def kernel(x: np.ndarray, y: np.ndarray) -> np.ndarray:
    loss, _ = _run(x, y)
    return np.asarray(loss, dtype=np.float32)


# revision 9
# speedup vs baseline: 6231.4617x; 6231.4617x over previous
"""Contrastive loss (ConStealLoss) kernel for 8 Trainium2 NeuronCores.

Computes, for x, y in R^[B=8192, D=128], T=0.07:
    sim1 = exp(x @ y.T / T); sim2 = exp(x @ x.T / T)
    l_neg_1 = rowsum(sim1) - diag(sim1); l_neg_2 = rowsum(sim2) - diag(sim2)
    l_pos = exp(rowsum(x*y) / T)
    loss = mean(-log(l_pos / (l_neg_1 + l_neg_2)))

Sharding: data-parallel over rows of x (1024 rows/core); y and x are
replicated as bf16 [D, B] "key" panels. Each core produces the partial
sum of log(l_pos/(l1+l2)) over its rows; the host combines.
"""

import numpy as np
import ml_dtypes

B = 8192
D = 128
TEMP = 0.07
NCORES = 8
R = B // NCORES            # 1024 rows per core
MT = R // 128              # 8 row-tiles of 128 per core
CHUNK = 2048               # keys per ScalarE exp instruction (4 PSUM banks)
NG = B // CHUNK            # 4 chunk groups over the key axis
NB = CHUNK // 512          # 4 matmuls (N=512) per chunk

_CACHE = {}


def _build_nc():
    import concourse.mybir as mybir
    import concourse.tile as tile
    from concourse import bacc

    f32 = mybir.dt.float32
    bf16 = mybir.dt.bfloat16
    AF = mybir.ActivationFunctionType
    AX = mybir.AxisListType
    ALU = mybir.AluOpType

    nc = bacc.Bacc("TRN2", debug=False)

    xT_d = nc.declare_dram_parameter("xT", [D, B], bf16, isOutput=False)
    yT_d = nc.declare_dram_parameter("yT", [D, B], bf16, isOutput=False)
    xcT_d = nc.declare_dram_parameter("xcT", [D, R], bf16, isOutput=False)
    xn_d = nc.declare_dram_parameter("xn", [R, D], f32, isOutput=False)
    yn_d = nc.declare_dram_parameter("yn", [R, D], f32, isOutput=False)
    out_d = nc.declare_dram_parameter("out", [128, 1], f32, isOutput=True)

    with tile.TileContext(nc) as tc:
        with (
            tc.tile_pool(name="const", bufs=1) as cpool,
            tc.tile_pool(name="scratch", bufs=3) as spool,
            tc.tile_pool(name="psum", bufs=2, space="PSUM") as ppool,
        ):
            # Chunked loads: one tile per 2048-key panel chunk so matmuls can
            # start as soon as their own chunk lands (DMA/compute overlap).
            xcT = cpool.tile([D, R], bf16)
            nc.sync.dma_start(xcT[:], xcT_d.ap())
            yTg, xTg = [], []
            for g in range(NG):
                t = cpool.tile([D, CHUNK], bf16, name=f"yT_{g}", tag=f"yT_{g}")
                nc.sync.dma_start(t[:], yT_d.ap()[:, g * CHUNK:(g + 1) * CHUNK])
                yTg.append(t)
            for g in range(NG):
                t = cpool.tile([D, CHUNK], bf16, name=f"xT_{g}", tag=f"xT_{g}")
                nc.sync.dma_start(t[:], xT_d.ap()[:, g * CHUNK:(g + 1) * CHUNK])
                xTg.append(t)
            xns, yns = [], []
            for m in range(MT):
                xnm = cpool.tile([128, D], f32, name=f"xn_{m}", tag=f"xn_{m}")
                ynm = cpool.tile([128, D], f32, name=f"yn_{m}", tag=f"yn_{m}")
                nc.sync.dma_start(xnm[:], xn_d.ap()[m * 128:(m + 1) * 128, :])
                nc.sync.dma_start(ynm[:], yn_d.ap()[m * 128:(m + 1) * 128, :])
                xns.append(xnm)
                yns.append(ynm)

            racc = cpool.tile([128, MT * 2 * NG], f32)
            d1 = cpool.tile([128, MT], f32)
            d2 = cpool.tile([128, MT], f32)

            # Per-row dots: d1[r] = <x_r, y_r>, d2[r] = |x_r|^2 (f32, natural layout)
            for m in range(MT):
                prod = spool.tile([128, D], f32, tag="prod")
                nc.vector.tensor_mul(prod[:], xns[m][:], yns[m][:])
                nc.vector.reduce_sum(d1[:, m:m + 1], prod[:], axis=AX.X)
                prod2 = spool.tile([128, D], f32, tag="prod")
                nc.vector.tensor_mul(prod2[:], xns[m][:], xns[m][:])
                nc.vector.reduce_sum(d2[:, m:m + 1], prod2[:], axis=AX.X)

            # Main: PE matmul -> PSUM [128, 2048]; ScalarE exp with row-sum accumulate
            for m in range(MT):
                lhsT = xcT[:, m * 128:(m + 1) * 128]
                for s, keys in enumerate((yTg, xTg)):
                    for g in range(NG):
                        pt = ppool.tile([128, CHUNK], f32, tag="pt")
                        for b in range(NB):
                            nc.tensor.matmul(
                                pt[:, b * 512:(b + 1) * 512],
                                lhsT,
                                keys[g][:, b * 512:(b + 1) * 512],
                                start=True, stop=True,
                            )
                        es = spool.tile([128, CHUNK], bf16, tag="es")
                        col = (m * 2 + s) * NG + g
                        nc.scalar.activation(
                            es[:], pt[:], AF.Exp, scale=1.0 / TEMP,
                            accum_out=racc[:, col:col + 1],
                        )

            # Row sums per (m, sim)
            rs1 = cpool.tile([128, MT], f32)
            rs2 = cpool.tile([128, MT], f32)
            for m in range(MT):
                c0 = (m * 2) * NG
                nc.vector.reduce_sum(rs1[:, m:m + 1], racc[:, c0:c0 + NG], axis=AX.X)
                nc.vector.reduce_sum(rs2[:, m:m + 1], racc[:, c0 + NG:c0 + 2 * NG], axis=AX.X)

            # Epilogue: l1 = rs1 - exp(d1/T); l2 = rs2 - exp(d2/T);
            # lg = log(exp(d1/T) / (l1 + l2)); out = rowwise sum of lg
            diag1 = cpool.tile([128, MT], f32)
            diag2 = cpool.tile([128, MT], f32)
            nc.scalar.activation(diag1[:], d1[:], AF.Exp, scale=1.0 / TEMP)
            nc.scalar.activation(diag2[:], d2[:], AF.Exp, scale=1.0 / TEMP)
            l1 = cpool.tile([128, MT], f32)
            l2 = cpool.tile([128, MT], f32)
            nc.vector.tensor_sub(l1[:], rs1[:], diag1[:])
            nc.vector.tensor_sub(l2[:], rs2[:], diag2[:])
            ssum = cpool.tile([128, MT], f32)
            nc.vector.tensor_add(ssum[:], l1[:], l2[:])
            rec = cpool.tile([128, MT], f32)
            nc.vector.reciprocal(rec[:], ssum[:])
            ratio = cpool.tile([128, MT], f32)
            nc.vector.tensor_mul(ratio[:], diag1[:], rec[:])
            lg = cpool.tile([128, MT], f32)
            nc.scalar.activation(lg[:], ratio[:], AF.Ln)
            rowpart = cpool.tile([128, 1], f32)
            nc.vector.reduce_sum(rowpart[:], lg[:], axis=AX.X)
            nc.sync.dma_start(out_d.ap(), rowpart[:])

    nc.compile()
    return nc


def _prep_inputs(x: np.ndarray, y: np.ndarray):
    x = np.ascontiguousarray(x, dtype=np.float32)
    y = np.ascontiguousarray(y, dtype=np.float32)
    xT = np.ascontiguousarray(x.T).astype(ml_dtypes.bfloat16)
    yT = np.ascontiguousarray(y.T).astype(ml_dtypes.bfloat16)
    in_maps = []
    for c in range(NCORES):
        sl = slice(c * R, (c + 1) * R)
        in_maps.append({
            "xT": xT,
            "yT": yT,
            "xcT": np.ascontiguousarray(xT[:, sl]),
            "xn": np.ascontiguousarray(x[sl]),
            "yn": np.ascontiguousarray(y[sl]),
        })
    return in_maps


def _get_runner():
    """Build the Bacc program once and wrap it in a cached sharded jit."""
    if "runner" in _CACHE:
        return _CACHE["runner"]
    import jax
    import jax.numpy as jnp
    from jax.sharding import Mesh, PartitionSpec
    from jax.experimental.shard_map import shard_map
    import concourse.mybir as mybir
    from concourse import bass2jax

    nc = _build_nc()
    bass2jax.install_neuronx_cc_hook()

    in_names = []
    out_names = []
    out_avals = []
    partition_name = (
        nc.partition_id_tensor.name if nc.partition_id_tensor else None
    )
    for alloc in nc.m.functions[0].allocations:
        if not isinstance(alloc, mybir.MemoryLocationSet):
            continue
        name = alloc.memorylocations[0].name
        if alloc.kind == "ExternalInput":
            if name != partition_name:
                in_names.append(name)
        elif alloc.kind == "ExternalOutput":
            out_names.append(name)
            out_avals.append(
                jax.core.ShapedArray(
                    tuple(alloc.tensor_shape), mybir.dt.np(alloc.dtype)
                )
            )
    n_params = len(in_names)
    n_outs = len(out_names)
    all_names = list(in_names) + list(out_names)
    if partition_name is not None:
        all_names.append(partition_name)

    def _body(*args):
        operands = list(args)
        if partition_name is not None:
            operands.append(bass2jax.partition_id_tensor())
        outs = bass2jax._bass_exec_p.bind(
            *operands,
            out_avals=tuple(out_avals),
            in_names=tuple(all_names),
            out_names=tuple(out_names),
            lowering_input_output_aliases=(),
            sim_require_finite=False,
            sim_require_nnan=False,
            nc=nc,
        )
        return tuple(outs)

    devices = jax.devices()[:NCORES]
    mesh = Mesh(np.asarray(devices), ("core",))
    in_specs = (PartitionSpec("core"),) * (n_params + n_outs)
    out_specs = (PartitionSpec("core"),) * n_outs
    donate = tuple(range(n_params, n_params + n_outs))
    jitted = jax.jit(
        shard_map(_body, mesh=mesh, in_specs=in_specs, out_specs=out_specs,
                  check_rep=False),
        donate_argnums=donate,
        keep_unused=True,
    )
    zero_shapes = [
        (tuple(av.shape), av.dtype) for av in out_avals
    ]
    runner = {
        "jitted": jitted,
        "in_names": in_names,
        "out_names": out_names,
        "zero_shapes": zero_shapes,
        "nc": nc,
    }
    _CACHE["runner"] = runner
    return runner


def _execute(in_maps):
    r = _get_runner()
    concat_in = [
        np.concatenate([np.asarray(in_maps[c][name]) for c in range(NCORES)], axis=0)
        for name in r["in_names"]
    ]
    zeros = [np.zeros((NCORES * s[0],) + tuple(s[1:]), dt)
             for s, dt in r["zero_shapes"]]
    outs = r["jitted"](*concat_in, *zeros)
    results = []
    for c in range(NCORES):
        m = {}
        for i, name in enumerate(r["out_names"]):
            arr = np.asarray(outs[i])
            per = arr.shape[0] // NCORES
            m[name] = arr[c * per:(c + 1) * per]
        results.append(m)
    return results


def _run(x: np.ndarray, y: np.ndarray, trace: bool = False):
    in_maps = _prep_inputs(x, y)
    results = _execute(in_maps)
    total = np.float32(0.0)
    for c in range(NCORES):
        total = total + results[c]["out"].astype(np.float32).sum()
    loss = np.float32(-(total / np.float32(B)))
    return loss, results


def kernel(x: np.ndarray, y: np.ndarray) -> np.ndarray:
    loss, _ = _run(x, y)
    return np.asarray(loss, dtype=np.float32)


# revision 18
# speedup vs baseline: 6297.4673x; 1.0106x over previous
"""Contrastive loss (ConStealLoss) kernel for 8 Trainium2 NeuronCores.

Computes, for x, y in R^[B=8192, D=128], T=0.07:
    sim1 = exp(x @ y.T / T); sim2 = exp(x @ x.T / T)
    l_neg_1 = rowsum(sim1) - diag(sim1); l_neg_2 = rowsum(sim2) - diag(sim2)
    l_pos = exp(rowsum(x*y) / T)
    loss = mean(-log(l_pos / (l_neg_1 + l_neg_2)))

Sharding: data-parallel over rows of x (1024 rows/core); y and x are
replicated as bf16 [D, B] "key" panels. Each core produces the partial
sum of log(l_pos/(l1+l2)) over its rows; the host combines.
"""

import numpy as np
import ml_dtypes

B = 8192
D = 128
TEMP = 0.07
NCORES = 8
R = B // NCORES            # 1024 rows per core
MT = R // 128              # 8 row-tiles of 128 per core
CHUNK = 2048               # keys per ScalarE exp instruction (4 PSUM banks)
NG = B // CHUNK            # 4 chunk groups over the key axis
NB = CHUNK // 512          # 4 matmuls (N=512) per chunk

_CACHE = {}


def _build_nc():
    import concourse.mybir as mybir
    import concourse.tile as tile
    from concourse import bacc

    f32 = mybir.dt.float32
    bf16 = mybir.dt.bfloat16
    AF = mybir.ActivationFunctionType
    AX = mybir.AxisListType
    ALU = mybir.AluOpType

    nc = bacc.Bacc("TRN2", debug=False)

    xT_d = nc.declare_dram_parameter("xT", [D, B], bf16, isOutput=False)
    yT_d = nc.declare_dram_parameter("yT", [D, B], bf16, isOutput=False)
    xcT_d = nc.declare_dram_parameter("xcT", [D, R], bf16, isOutput=False)
    xn_d = nc.declare_dram_parameter("xn", [R, D], f32, isOutput=False)
    yn_d = nc.declare_dram_parameter("yn", [R, D], f32, isOutput=False)
    out_d = nc.declare_dram_parameter("out", [128, MT], f32, isOutput=True)

    with tile.TileContext(nc) as tc:
        with (
            tc.tile_pool(name="const", bufs=1) as cpool,
            tc.tile_pool(name="scratch", bufs=3) as spool,
            tc.tile_pool(name="dead", bufs=1) as dpool,
            tc.tile_pool(name="psum", bufs=2, space="PSUM") as ppool,
        ):
            # Chunked loads: one tile per 2048-key panel chunk so matmuls can
            # start as soon as their own chunk lands (DMA/compute overlap).
            xcT = cpool.tile([D, R], bf16)
            nc.sync.dma_start(xcT[:], xcT_d.ap())
            yTg, xTg = [], []
            for g in range(NG):
                t = cpool.tile([D, CHUNK], bf16, name=f"yT_{g}", tag=f"yT_{g}")
                nc.sync.dma_start(t[:], yT_d.ap()[:, g * CHUNK:(g + 1) * CHUNK])
                yTg.append(t)
            for g in range(NG):
                t = cpool.tile([D, CHUNK], bf16, name=f"xT_{g}", tag=f"xT_{g}")
                nc.sync.dma_start(t[:], xT_d.ap()[:, g * CHUNK:(g + 1) * CHUNK])
                xTg.append(t)
            # xn/yn feed only the per-row dot products (epilogue inputs), so
            # they load after the key panels that gate the matmul pipeline.
            xns, yns = [], []
            for m in range(MT):
                xnm = cpool.tile([128, D], f32, name=f"xn_{m}", tag=f"xn_{m}")
                ynm = cpool.tile([128, D], f32, name=f"yn_{m}", tag=f"yn_{m}")
                nc.sync.dma_start(xnm[:], xn_d.ap()[m * 128:(m + 1) * 128, :])
                nc.sync.dma_start(ynm[:], yn_d.ap()[m * 128:(m + 1) * 128, :])
                xns.append(xnm)
                yns.append(ynm)

            racc = cpool.tile([128, MT * 2 * NG], f32)
            d1 = cpool.tile([128, MT], f32)
            d2 = cpool.tile([128, MT], f32)

            # Per-row dots: d1[r] = <x_r, y_r>, d2[r] = |x_r|^2 (f32, natural layout)
            for m in range(MT):
                prod = spool.tile([128, D], f32, tag="prod")
                nc.vector.tensor_mul(prod[:], xns[m][:], yns[m][:])
                nc.vector.reduce_sum(d1[:, m:m + 1], prod[:], axis=AX.X)
                prod2 = spool.tile([128, D], f32, tag="prod")
                nc.vector.tensor_mul(prod2[:], xns[m][:], xns[m][:])
                nc.vector.reduce_sum(d2[:, m:m + 1], prod2[:], axis=AX.X)

            # Diagonal exps early: they depend only on d1/d2, so placing them
            # before the main loop keeps them off the ACT queue's tail.
            diag1 = cpool.tile([128, MT], f32)
            diag2 = cpool.tile([128, MT], f32)
            nc.scalar.activation(diag1[:], d1[:], AF.Exp, scale=1.0 / TEMP)
            nc.scalar.activation(diag2[:], d2[:], AF.Exp, scale=1.0 / TEMP)

            # Main: PE matmul -> PSUM [128, 2048]; ScalarE exp with row-sum accumulate
            for m in range(MT):
                lhsT = xcT[:, m * 128:(m + 1) * 128]
                for s, keys in enumerate((yTg, xTg)):
                    for g in range(NG):
                        pt = ppool.tile([128, CHUNK], f32, tag="pt")
                        for b in range(NB):
                            nc.tensor.matmul(
                                pt[:, b * 512:(b + 1) * 512],
                                lhsT,
                                keys[g][:, b * 512:(b + 1) * 512],
                                start=True, stop=True,
                            )
                        es = spool.tile([128, CHUNK], bf16, tag="es")
                        col = (m * 2 + s) * NG + g
                        nc.scalar.activation(
                            es[:], pt[:], AF.Exp, scale=1.0 / TEMP,
                            accum_out=racc[:, col:col + 1],
                        )

            # Per-m epilogue: l1 = rs1 - exp(d1/T); l2 = rs2 - exp(d2/T);
            # ratio = exp(d1/T) / (l1 + l2). Per-m granularity keeps all but
            # the last m-tile's chain off the kernel tail. The final
            # elementwise log + mean happen in the host gather (keeps ACT on
            # one table set — the Ln load would otherwise sit on the tail).
            rs1 = cpool.tile([128, MT], f32)
            rs2 = cpool.tile([128, MT], f32)
            ratio = cpool.tile([128, MT], f32)
            for m in range(MT):
                c0 = (m * 2) * NG
                nc.vector.reduce_sum(rs1[:, m:m + 1], racc[:, c0:c0 + NG], axis=AX.X)
                nc.vector.reduce_sum(rs2[:, m:m + 1], racc[:, c0 + NG:c0 + 2 * NG], axis=AX.X)
                l1m = spool.tile([128, 1], f32, tag="l1m")
                l2m = spool.tile([128, 1], f32, tag="l2m")
                nc.vector.tensor_sub(l1m[:], rs1[:, m:m + 1], diag1[:, m:m + 1])
                nc.vector.tensor_sub(l2m[:], rs2[:, m:m + 1], diag2[:, m:m + 1])
                ssm = spool.tile([128, 1], f32, tag="ssm")
                nc.vector.tensor_add(ssm[:], l1m[:], l2m[:])
                recm = spool.tile([128, 1], f32, tag="recm")
                nc.vector.reciprocal(recm[:], ssm[:])
                nc.vector.tensor_mul(ratio[:, m:m + 1], diag1[:, m:m + 1], recm[:])
            nc.sync.dma_start(out_d.ap(), ratio[:])

    nc.compile()
    return nc


def _prep_inputs(x: np.ndarray, y: np.ndarray):
    x = np.ascontiguousarray(x, dtype=np.float32)
    y = np.ascontiguousarray(y, dtype=np.float32)
    xT = np.ascontiguousarray(x.T).astype(ml_dtypes.bfloat16)
    yT = np.ascontiguousarray(y.T).astype(ml_dtypes.bfloat16)
    in_maps = []
    for c in range(NCORES):
        sl = slice(c * R, (c + 1) * R)
        in_maps.append({
            "xT": xT,
            "yT": yT,
            "xcT": np.ascontiguousarray(xT[:, sl]),
            "xn": np.ascontiguousarray(x[sl]),
            "yn": np.ascontiguousarray(y[sl]),
        })
    return in_maps


def _get_runner():
    """Build the Bacc program once and wrap it in a cached sharded jit."""
    if "runner" in _CACHE:
        return _CACHE["runner"]
    import jax
    import jax.numpy as jnp
    from jax.sharding import Mesh, PartitionSpec
    from jax.experimental.shard_map import shard_map
    import concourse.mybir as mybir
    from concourse import bass2jax

    nc = _build_nc()
    bass2jax.install_neuronx_cc_hook()

    in_names = []
    out_names = []
    out_avals = []
    partition_name = (
        nc.partition_id_tensor.name if nc.partition_id_tensor else None
    )
    for alloc in nc.m.functions[0].allocations:
        if not isinstance(alloc, mybir.MemoryLocationSet):
            continue
        name = alloc.memorylocations[0].name
        if alloc.kind == "ExternalInput":
            if name != partition_name:
                in_names.append(name)
        elif alloc.kind == "ExternalOutput":
            out_names.append(name)
            out_avals.append(
                jax.core.ShapedArray(
                    tuple(alloc.tensor_shape), mybir.dt.np(alloc.dtype)
                )
            )
    n_params = len(in_names)
    n_outs = len(out_names)
    all_names = list(in_names) + list(out_names)
    if partition_name is not None:
        all_names.append(partition_name)

    def _body(*args):
        operands = list(args)
        if partition_name is not None:
            operands.append(bass2jax.partition_id_tensor())
        outs = bass2jax._bass_exec_p.bind(
            *operands,
            out_avals=tuple(out_avals),
            in_names=tuple(all_names),
            out_names=tuple(out_names),
            lowering_input_output_aliases=(),
            sim_require_finite=False,
            sim_require_nnan=False,
            nc=nc,
        )
        return tuple(outs)

    devices = jax.devices()[:NCORES]
    mesh = Mesh(np.asarray(devices), ("core",))
    in_specs = (PartitionSpec("core"),) * (n_params + n_outs)
    out_specs = (PartitionSpec("core"),) * n_outs
    donate = tuple(range(n_params, n_params + n_outs))
    jitted = jax.jit(
        shard_map(_body, mesh=mesh, in_specs=in_specs, out_specs=out_specs,
                  check_rep=False),
        donate_argnums=donate,
        keep_unused=True,
    )
    zero_shapes = [
        (tuple(av.shape), av.dtype) for av in out_avals
    ]
    runner = {
        "jitted": jitted,
        "in_names": in_names,
        "out_names": out_names,
        "zero_shapes": zero_shapes,
        "nc": nc,
    }
    _CACHE["runner"] = runner
    return runner


def _execute(in_maps):
    r = _get_runner()
    concat_in = [
        np.concatenate([np.asarray(in_maps[c][name]) for c in range(NCORES)], axis=0)
        for name in r["in_names"]
    ]
    zeros = [np.zeros((NCORES * s[0],) + tuple(s[1:]), dt)
             for s, dt in r["zero_shapes"]]
    outs = r["jitted"](*concat_in, *zeros)
    results = []
    for c in range(NCORES):
        m = {}
        for i, name in enumerate(r["out_names"]):
            arr = np.asarray(outs[i])
            per = arr.shape[0] // NCORES
            m[name] = arr[c * per:(c + 1) * per]
        results.append(m)
    return results


def _run(x: np.ndarray, y: np.ndarray, trace: bool = False):
    in_maps = _prep_inputs(x, y)
    results = _execute(in_maps)
    total = np.float32(0.0)
    with np.errstate(divide="ignore", invalid="ignore"):
        for c in range(NCORES):
            ratio = results[c]["out"].astype(np.float32)
            total = total + np.log(ratio).sum(dtype=np.float32)
    loss = np.float32(-(total / np.float32(B)))
    return loss, results


def kernel(x: np.ndarray, y: np.ndarray) -> np.ndarray:
    loss, _ = _run(x, y)
    return np.asarray(loss, dtype=np.float32)
